# revision 1
# baseline (speedup 1.0000x reference)
"""Complex-magnitude MaxPool2d (k=2, s=2) Trainium2 Bass kernel.

Input  x:  [16, 2, 64, 224, 224] f32  (plane 0 = real, plane 1 = imag)
Output:    [16, 2, 64, 112, 112] f32  (value of the window element with the
                                       largest |z|^2 = re^2 + im^2)

Sharding: pure data parallel over batch: 16 / 8 cores = 2 examples per core.
Per core the 2(batch) x 64(channel) = 128 image planes map 1:1 onto the 128
SBUF partitions; DMA moves 28 image rows at a time in a single 128-partition
dma_start (one transfer spans all 16 SBUF AXI ports and amortizes the ~2us
per-dma fixed cost); compute runs on 14-row subchunks.

Selection reproduces jnp.argmax's first-index tie-break exactly:
horizontal pass first (left/even column wins ties via is_ge), then vertical
(top row wins ties).  norm2 = fl(fl(re*re)+fl(im*im)) in f32 — ACT's Square
activation and GPSIMD's f32 add are bit-exact with the reference expression
(hardware-verified), so selections match the reference everywhere,
including exact ties.

Engine split (measured rates):
  ScalarE : squares (one ACT op per subchunk), select pre-fill copies
  GPSIMD  : norm add (in place over the squares)
  VectorE : is_ge masks + copy_predicated selects.  Masks and predicated
            dst stay contiguous (2x faster than strided), and each pred
            selects re+im together via a step-0 broadcast mask.
  DMA     : 128-partition transfers; outputs staged to long runs.
"""

import numpy as np

import concourse.bass as bass
import concourse.mybir as mybir
from concourse import bacc, bass_utils, tile

# Per-core shard geometry (hardcoded; kernel.py must be self-contained).
NCORES = 8
B = 2            # batch per core
RI = 2           # real/imag planes
C = 64           # channels
H = W = 224
HO, WO = H // 2, W // 2
P = 128          # SBUF partitions = B * C
RD = 28          # image rows per DMA chunk
R = 14           # image rows per compute subchunk
SUB = RD // R    # compute subchunks per DMA chunk (2)
NCHUNK = H // RD  # 8
N = R * W        # free elements per plane per subchunk (3136)
GROUP = 4        # subchunks staged per output store (28 output rows)
SROWS = GROUP * (R // 2)

F32 = mybir.dt.float32
I8 = mybir.dt.uint8
OP = mybir.AluOpType
ACTF = mybir.ActivationFunctionType

_NC_CACHE = []


def _build_nc() -> bass.Bass:
    nc = bacc.Bacc("TRN2", target_bir_lowering=False, debug=False)
    # host pre-transposed: partition-major [b*c, ri, H, W] so every DMA is a
    # single-dim 128-partition transfer (hits all 16 SBUF AXI ports)
    x = nc.dram_tensor("x", [P, RI, H, W], F32, kind="ExternalInput").ap()
    out = nc.dram_tensor("out", [P, RI, HO, WO], F32, kind="ExternalOutput").ap()

    with tile.TileContext(nc) as tc:
        with tc.tile_pool(name="pool", bufs=2) as pool:
            stage = None
            subidx = 0
            for k in range(NCHUNK):
                r0 = k * RD
                # xri free layout per partition: [ri][row 0..RD)[col]
                xri = pool.tile([P, RI * RD * W], F32, tag="xri")
                nrw = RD * W
                nc.sync.dma_start(
                    out=xri.rearrange("p (ri f) -> p ri f", ri=RI),
                    in_=x[:, :, r0 : r0 + RD, :].rearrange("p ri r w -> p ri (r w)"),
                )

                for s in range(SUB):
                    # subchunk views: rows rs..rs+R of each plane
                    xri6 = xri.rearrange(
                        "p (ri r w t) -> p ri r w t", ri=RI, r=RD, w=WO, t=2
                    )[:, :, s * R : (s + 1) * R, :, :]

                    # squares of re+im rows in one ACT op; norm2 in place
                    # over the re half; im half is reused as riH below
                    sqri = pool.tile([P, RI * N], F32, tag="sqri")
                    nc.scalar.activation(
                        out=sqri.rearrange(
                            "p (ri r w t) -> p ri r w t", ri=RI, r=R, w=WO, t=2
                        ),
                        in_=xri6,
                        func=ACTF.Square,
                    )
                    # norm add on DVE: GPSIMD shares a SBUF read port with
                    # DVE 2-stream ops and stalls them 2x when overlapped,
                    # so keeping GPSIMD idle is a net win
                    nrm = sqri[:, :N]
                    nc.vector.tensor_tensor(
                        out=nrm, in0=nrm, in1=sqri[:, N:], op=OP.add
                    )

                    nrm4 = nrm.rearrange("p (r w t) -> p r w t", r=R, w=WO, t=2)
                    nE, nO = nrm4[:, :, :, 0], nrm4[:, :, :, 1]

                    # horizontal mask (contiguous u8): even/left wins ties
                    cH = pool.tile([P, R * WO], I8, tag="cH")
                    cH3 = cH.rearrange("p (r w) -> p r w", r=R, w=WO)
                    nc.vector.tensor_tensor(out=cH3, in0=nE, in1=nO, op=OP.is_ge)
                    # horizontal norm max -> nrm odd slots (in place)
                    nc.vector.tensor_tensor(out=nO, in0=nE, in1=nO, op=OP.max)

                    # horizontal select of (re, im) together into the dead
                    # im-squares half: pre-fill with odd/right, overwrite
                    # where cH
                    riH = sqri[:, N:]
                    riH4 = riH.rearrange("p (ri r w) -> p ri r w", ri=RI, r=R, w=WO)
                    nc.scalar.copy(out=riH4, in_=xri6[:, :, :, :, 1])
                    cHb = cH3.unsqueeze(1).broadcast_to([P, RI, R, WO])
                    nc.vector.copy_predicated(
                        out=riH4, mask=cHb, data=xri6[:, :, :, :, 0]
                    )

                    # vertical mask from the horizontal maxes: top wins ties
                    nrm5 = nrm.rearrange(
                        "p (rp rt w t) -> p rp rt w t", rp=R // 2, rt=2, w=WO, t=2
                    )
                    cV = pool.tile([P, (R // 2) * WO], I8, tag="cV")
                    cV3 = cV.rearrange("p (rp w) -> p rp w", rp=R // 2, w=WO)
                    nc.vector.tensor_tensor(
                        out=cV3,
                        in0=nrm5[:, :, 0, :, 1],
                        in1=nrm5[:, :, 1, :, 1],
                        op=OP.is_ge,
                    )

                    # vertical select into the staged output tile
                    riH5 = riH.rearrange(
                        "p (ri rp rt w) -> p ri rp rt w",
                        ri=RI, rp=R // 2, rt=2, w=WO,
                    )
                    if subidx % GROUP == 0:
                        stage = pool.tile([P, RI * SROWS * WO], F32, tag="stage")
                    stage4 = stage.rearrange(
                        "p (ri r w) -> p ri r w", ri=RI, r=SROWS, w=WO
                    )
                    s0 = (subidx % GROUP) * (R // 2)
                    dst = stage4[:, :, s0 : s0 + R // 2, :]
                    nc.scalar.copy(out=dst, in_=riH5[:, :, :, 1, :])
                    cVb = cV3.unsqueeze(1).broadcast_to([P, RI, R // 2, WO])
                    nc.vector.copy_predicated(
                        out=dst, mask=cVb, data=riH5[:, :, :, 0, :]
                    )

                    if (subidx + 1) % GROUP == 0:
                        g0 = (subidx + 1 - GROUP) * (R // 2)
                        nc.sync.dma_start(
                            out=out[:, :, g0 : g0 + SROWS, :].rearrange(
                                "p ri r w -> p ri (r w)"
                            ),
                            in_=stage.rearrange("p (ri f) -> p ri f", ri=RI),
                        )
                    subidx += 1
    nc.compile()
    return nc


def get_nc() -> bass.Bass:
    if not _NC_CACHE:
        _NC_CACHE.append(_build_nc())
    return _NC_CACHE[0]


def kernel(x: np.ndarray, **run_kwargs) -> np.ndarray:
    nc = get_nc()
    xs = np.asarray(x, dtype=np.float32)
    assert xs.shape == (NCORES * B, RI, C, H, W), xs.shape
    # [16,2,64,H,W] -> per core [b,c,ri,H,W] flattened to [128,ri,H,W]
    xt = np.ascontiguousarray(xs.transpose(0, 2, 1, 3, 4))
    in_maps = [
        {"x": xt[B * i : B * (i + 1)].reshape(P, RI, H, W)} for i in range(NCORES)
    ]
    res = bass_utils.run_bass_kernel_spmd(
        nc, in_maps, core_ids=list(range(NCORES)), **run_kwargs
    )
    # per-core [128,ri,HO,WO] -> [b,c,ri,HO,WO] -> [b,ri,c,HO,WO]
    out = np.concatenate(
        [
            res.results[i]["out"].reshape(B, C, RI, HO, WO).transpose(0, 2, 1, 3, 4)
            for i in range(NCORES)
        ],
        axis=0,
    )
    if run_kwargs:
        kernel.last_results = res
    return np.ascontiguousarray(out)



# revision 4
# speedup vs baseline: 1.1926x; 1.1926x over previous
"""Complex-magnitude MaxPool2d (k=2, s=2) Trainium2 Bass kernel.

Input  x:  [16, 2, 64, 224, 224] f32  (plane 0 = real, plane 1 = imag)
Output:    [16, 2, 64, 112, 112] f32  (value of the window element with the
                                       largest |z|^2 = re^2 + im^2)

Sharding: pure data parallel over batch: 16 / 8 cores = 2 examples per core.
Per core the 2(batch) x 64(channel) = 128 image planes map 1:1 onto the 128
SBUF partitions; DMA moves 14 image rows per chunk (16 chunks), software-
pipelined two-deep so DVE never waits on DMA or ACT.

DVE work is compressed with custom fused DVE ops (per-NEFF uop table):
  SQADD   nrm  = re^2 + im^2                  (one pass; kills the ACT Square
                                               pass and the separate DVE add)
  SIGNSEL smax = sel(nE>=nO, -nE, nO)         (H-compare and H-max in one op:
                                               sign bit = "left/even wins",
                                               magnitude = winning norm)
  SQGE    cV   = sq(smaxT) >= sq(smaxB)       (V-compare on |smax| via squares)

The H-select mask (u8, nonzero = even/left wins <=> smax < 0) is derived on
the idle ScalarE: Sign(-smax) in {-1,0,+1} then Relu -> {0,1}, exact in u8.
Selection reproduces jnp.argmax's first-index tie-break: left wins H ties
(is_ge), top wins V ties; norms computed as fl(fl(re^2)+fl(im^2)) on IEEE f32
ALUs, bit-identical to the reference.

Engine split per chunk: DVE: SQADD, SIGNSEL, SQGE, predicated H/V selects.
ScalarE: H prefill (odd cols), mask extract, V prefill (bottom rows).
Pipeline skew: predH runs one chunk behind, predV two behind, so every DVE op
only depends on work finished in earlier iterations.
"""

import re as _re

import numpy as np

import concourse.bass as bass
import concourse.mybir as mybir
from concourse import bacc, bass_utils, tile
from concourse import dve_ops as _dvo
from concourse.dve_spec import Spec as _Spec, Src0 as _S0, Src1 as _S1
from concourse.dve_spec import sq as _sq, select as _sel

# Per-core shard geometry (hardcoded; kernel.py must be self-contained).
NCORES = 8
B = 2            # batch per core
RI = 2           # real/imag planes
C = 64           # channels
H = W = 224
HO, WO = H // 2, W // 2
P = 128          # SBUF partitions = B * C
RD = 14          # image rows per DMA chunk == compute chunk
NCHUNK = H // RD  # 16
RO = RD // 2     # output rows per chunk (7)
N = RD * W       # free elements per plane per chunk (3136)
GROUP = 2        # chunks per output store (14 output rows)

F32 = mybir.dt.float32
U8 = mybir.dt.uint8
ACTF = mybir.ActivationFunctionType


def _reg(name, spec):
    """Register a custom DVE op, self-pinning its uops sha."""
    for o in _dvo.OPS:
        if o.name == name:
            return o
    op = _dvo.DveOp(name=name, spec=spec, subdim=False, uops_sha={})
    _dvo.OPS.append(op)
    _dvo.CUSTOM_DVE_SPECS[name] = spec
    _dvo._SUB_OPCODE_FOR_NAME[name] = _dvo._CUSTOM_DVE_ROW_BASE + len(_dvo.OPS) - 1
    assert max(_dvo._SUB_OPCODE_FOR_NAME.values()) < 0x20
    for ver in ("v3", "v4"):
        try:
            op.compile(ver)
        except ValueError as e:
            m = _re.search(r'uops_sha\["' + ver + r'"\]="([0-9a-f]+)"', str(e))
            if not m:
                raise
            op.uops_sha[ver] = m.group(1)
            op.compile(ver)
    return op


SQADD = _reg(
    "ANT_MP_SQADD",
    _Spec(
        body=_sq(_S0) + _sq(_S1),
        reference=lambda in0, in1, s0, s1, imm2: (
            in0.astype(np.float32) * in0 + in1.astype(np.float32) * in1
        ),
    ),
)
SIGNSEL = _reg(
    "ANT_MP_SIGNSEL",
    _Spec(
        body=_sel(_S0 >= _S1, -_S0, _S1),
        reference=lambda in0, in1, s0, s1, imm2: np.where(
            in0 >= in1, -in0, in1
        ).astype(np.float32),
    ),
)
SQGE = _reg(
    "ANT_MP_SQGE",
    _Spec(
        body=_sq(_S0) >= _sq(_S1),
        reference=lambda in0, in1, s0, s1, imm2: (
            in0.astype(np.float32) * in0 >= in1.astype(np.float32) * in1
        ).astype(np.float32),
    ),
)

_NC_CACHE = []


def _build_nc() -> bass.Bass:
    nc = bacc.Bacc("TRN2", target_bir_lowering=False, debug=False)
    # host pre-transposed: partition-major [b*c, ri, H, W] so every DMA is a
    # single-dim 128-partition transfer (hits all 16 SBUF AXI ports)
    x = nc.dram_tensor("x", [P, RI, H, W], F32, kind="ExternalInput").ap()
    out = nc.dram_tensor("out", [P, RI, HO, WO], F32, kind="ExternalOutput").ap()

    with tile.TileContext(nc) as tc:
        with tc.tile_pool(name="p3", bufs=3) as p3, \
             tc.tile_pool(name="p2", bufs=2) as p2, \
             tc.tile_pool(name="p1", bufs=1) as p1:

            xri_t, riH_t, cV_t, mH_t, stage_t = {}, {}, {}, {}, {}

            def dma_in(k):
                xri = p3.tile([P, RI * N], F32, tag="xri")
                xri_t[k] = xri
                nc.sync.dma_start(
                    out=xri.rearrange("p (ri f) -> p ri f", ri=RI),
                    in_=x[:, :, k * RD : (k + 1) * RD, :].rearrange(
                        "p ri r w -> p ri (r w)"
                    ),
                )

            def xri6(k):
                return xri_t[k].rearrange(
                    "p (ri r w t) -> p ri r w t", ri=RI, r=RD, w=WO, t=2
                )

            def riH4(k):
                return riH_t[k].rearrange(
                    "p (ri r w) -> p ri r w", ri=RI, r=RD, w=WO
                )

            def riH5(k):
                return riH_t[k].rearrange(
                    "p (ri rp rt w) -> p ri rp rt w", ri=RI, rp=RO, rt=2, w=WO
                )

            def stage_dst4(k):
                g = k % GROUP
                return stage_t[k // GROUP].rearrange(
                    "p (ri r w) -> p ri r w", ri=RI, r=GROUP * RO, w=WO
                )[:, :, g * RO : (g + 1) * RO, :]

            for k in range(NCHUNK):
                if k == 0:
                    dma_in(0)
                if k + 1 < NCHUNK:
                    dma_in(k + 1)

                # ACT: prefill H-losers (odd cols) straight from the input
                riH = p3.tile([P, RI * RD * WO], F32, tag="riH")
                riH_t[k] = riH
                nc.scalar.copy(out=riH4(k), in_=xri6(k)[:, :, :, :, 1])

                # DVE: nrm = re^2 + im^2 (fused, one pass over the chunk)
                nrm = p1.tile([P, N], F32, tag="nrm")
                xrr = xri_t[k].rearrange("p (ri f) -> p ri f", ri=RI)
                nc.vector._custom_dve(
                    SQADD, out=nrm[:, :], in0=xrr[:, 0, :], in1=xrr[:, 1, :]
                )

                # DVE: smax = sel(nE>=nO, -nE, nO): sign=mask, |.|=H-max
                smax = p2.tile([P, RD * WO], F32, tag="smax")
                nrm_t = nrm.rearrange("p (x t) -> p x t", t=2)
                nc.vector._custom_dve(
                    SIGNSEL, out=smax[:, :], in0=nrm_t[:, :, 0], in1=nrm_t[:, :, 1]
                )

                # ACT: mH u8 = relu(sign(-smax)) in {0,1}; 1 <=> even/left won
                mHs = p1.tile([P, RD * WO], F32, tag="mHs")
                nc.scalar.activation(out=mHs, in_=smax, func=ACTF.Sign, scale=-1.0)
                mH = p2.tile([P, RD * WO], U8, tag="mH")
                mH_t[k] = mH
                nc.scalar.activation(out=mH, in_=mHs, func=ACTF.Relu)

                # DVE: cV = sq(smaxT) >= sq(smaxB)  (|smaxT| >= |smaxB|)
                cV = p3.tile([P, RO * WO], U8, tag="cV")
                cV_t[k] = cV
                sm5 = smax.rearrange(
                    "p (rp rt w) -> p rp rt w", rp=RO, rt=2, w=WO
                )
                cv3 = cV.rearrange("p (r w) -> p r w", r=RO, w=WO)
                nc.vector._custom_dve(
                    SQGE, out=cv3, in0=sm5[:, :, 0, :], in1=sm5[:, :, 1, :]
                )

                def predh(j):
                    mb = mH_t[j].rearrange(
                        "p (r w) -> p r w", r=RD, w=WO
                    ).unsqueeze(1).broadcast_to([P, RI, RD, WO])
                    nc.vector.copy_predicated(
                        out=riH4(j), mask=mb, data=xri6(j)[:, :, :, :, 0]
                    )
                    del xri_t[j], mH_t[j]

                def vpre(j):
                    if j % GROUP == 0:
                        stage_t[j // GROUP] = p2.tile(
                            [P, RI * GROUP * RO * WO], F32, tag="stage",
                            name="stage",
                        )
                    nc.scalar.copy(out=stage_dst4(j), in_=riH5(j)[:, :, :, 1, :])

                def predv(j):
                    cb = cV_t[j].rearrange(
                        "p (r w) -> p r w", r=RO, w=WO
                    ).unsqueeze(1).broadcast_to([P, RI, RO, WO])
                    nc.vector.copy_predicated(
                        out=stage_dst4(j), mask=cb, data=riH5(j)[:, :, :, 0, :]
                    )
                    del riH_t[j], cV_t[j]
                    if j % GROUP == GROUP - 1:
                        jj = j // GROUP
                        nc.sync.dma_start(
                            out=out[
                                :, :, jj * GROUP * RO : (jj + 1) * GROUP * RO, :
                            ].rearrange("p ri r w -> p ri (r w)"),
                            in_=stage_t[jj].rearrange("p (ri f) -> p ri f", ri=RI),
                        )
                        del stage_t[jj]

                if k >= 1:
                    predh(k - 1)
                    vpre(k - 1)
                if k >= 2:
                    predv(k - 2)

            predh(NCHUNK - 1)
            vpre(NCHUNK - 1)
            predv(NCHUNK - 2)
            predv(NCHUNK - 1)
    nc.compile()
    return nc


def get_nc() -> bass.Bass:
    if not _NC_CACHE:
        _NC_CACHE.append(_build_nc())
    return _NC_CACHE[0]


def kernel(x: np.ndarray, **run_kwargs) -> np.ndarray:
    nc = get_nc()
    xs = np.asarray(x, dtype=np.float32)
    assert xs.shape == (NCORES * B, RI, C, H, W), xs.shape
    # [16,2,64,H,W] -> per core [b,c,ri,H,W] flattened to [128,ri,H,W]
    xt = np.ascontiguousarray(xs.transpose(0, 2, 1, 3, 4))
    in_maps = [
        {"x": xt[B * i : B * (i + 1)].reshape(P, RI, H, W)} for i in range(NCORES)
    ]
    res = bass_utils.run_bass_kernel_spmd(
        nc, in_maps, core_ids=list(range(NCORES)), **run_kwargs
    )
    # per-core [128,ri,HO,WO] -> [b,c,ri,HO,WO] -> [b,ri,c,HO,WO]
    out = np.concatenate(
        [
            res.results[i]["out"].reshape(B, C, RI, HO, WO).transpose(0, 2, 1, 3, 4)
            for i in range(NCORES)
        ],
        axis=0,
    )
    if run_kwargs:
        kernel.last_results = res
    return np.ascontiguousarray(out)


# revision 6
# speedup vs baseline: 1.2181x; 1.0214x over previous
"""Complex-magnitude MaxPool2d (k=2, s=2) Trainium2 Bass kernel.

Input  x:  [16, 2, 64, 224, 224] f32  (plane 0 = real, plane 1 = imag)
Output:    [16, 2, 64, 112, 112] f32  (value of the window element with the
                                       largest |z|^2 = re^2 + im^2)

Sharding: pure data parallel over batch: 16 / 8 cores = 2 examples per core.
Per core the 2(batch) x 64(channel) = 128 image planes map 1:1 onto the 128
SBUF partitions; DMA moves 14 image rows per chunk (16 chunks), software-
pipelined two-deep so DVE never waits on DMA or ACT.

DVE work is compressed with custom fused DVE ops (per-NEFF uop table):
  SQADD   nrm  = re^2 + im^2                  (one pass; kills the ACT Square
                                               pass and the separate DVE add)
  SIGNSEL smax = sel(nE>=nO, -nE, nO)         (H-compare and H-max in one op:
                                               sign bit = "left/even wins",
                                               magnitude = winning norm)
  SQGE    cV   = sq(smaxT) >= sq(smaxB)       (V-compare on |smax| via squares)

The H-select mask (u8, nonzero = even/left wins <=> smax < 0) is derived on
the idle ScalarE: Sign(-smax) in {-1,0,+1} then Relu -> {0,1}, exact in u8.
Selection reproduces jnp.argmax's first-index tie-break: left wins H ties
(is_ge), top wins V ties; norms computed as fl(fl(re^2)+fl(im^2)) on IEEE f32
ALUs, bit-identical to the reference.

Engine split per chunk: DVE: SQADD, SIGNSEL, SQGE, predicated H/V selects.
ScalarE: H prefill (odd cols), mask extract, V prefill (bottom rows).
Pipeline skew: predH runs one chunk behind, predV two behind, so every DVE op
only depends on work finished in earlier iterations.
"""

import re as _re

import numpy as np

import concourse.bass as bass
import concourse.mybir as mybir
from concourse import bacc, bass_utils, tile
from concourse import dve_ops as _dvo
from concourse.dve_spec import Spec as _Spec, Src0 as _S0, Src1 as _S1
from concourse.dve_spec import sq as _sq, select as _sel

# Per-core shard geometry (hardcoded; kernel.py must be self-contained).
NCORES = 8
B = 2            # batch per core
RI = 2           # real/imag planes
C = 64           # channels
H = W = 224
HO, WO = H // 2, W // 2
P = 128          # SBUF partitions = B * C
RD = 14          # image rows per DMA chunk == compute chunk
NCHUNK = H // RD  # 16
RO = RD // 2     # output rows per chunk (7)
N = RD * W       # free elements per plane per chunk (3136)
GROUP = 2        # chunks per output store (14 output rows)

F32 = mybir.dt.float32
U8 = mybir.dt.uint8
ACTF = mybir.ActivationFunctionType


def _reg(name, spec):
    """Register a custom DVE op, self-pinning its uops sha."""
    for o in _dvo.OPS:
        if o.name == name:
            return o
    op = _dvo.DveOp(name=name, spec=spec, subdim=False, uops_sha={})
    _dvo.OPS.append(op)
    _dvo.CUSTOM_DVE_SPECS[name] = spec
    _dvo._SUB_OPCODE_FOR_NAME[name] = _dvo._CUSTOM_DVE_ROW_BASE + len(_dvo.OPS) - 1
    assert max(_dvo._SUB_OPCODE_FOR_NAME.values()) < 0x20
    for ver in ("v3", "v4"):
        try:
            op.compile(ver)
        except ValueError as e:
            m = _re.search(r'uops_sha\["' + ver + r'"\]="([0-9a-f]+)"', str(e))
            if not m:
                raise
            op.uops_sha[ver] = m.group(1)
            op.compile(ver)
    return op


SQADD = _reg(
    "ANT_MP_SQADD",
    _Spec(
        body=_sq(_S0) + _sq(_S1),
        reference=lambda in0, in1, s0, s1, imm2: (
            in0.astype(np.float32) * in0 + in1.astype(np.float32) * in1
        ),
    ),
)
SIGNSEL = _reg(
    "ANT_MP_SIGNSEL",
    _Spec(
        body=_sel(_S0 >= _S1, -_S0, _S1),
        reference=lambda in0, in1, s0, s1, imm2: np.where(
            in0 >= in1, -in0, in1
        ).astype(np.float32),
    ),
)
SQGE = _reg(
    "ANT_MP_SQGE",
    _Spec(
        body=_sq(_S0) >= _sq(_S1),
        reference=lambda in0, in1, s0, s1, imm2: (
            in0.astype(np.float32) * in0 >= in1.astype(np.float32) * in1
        ).astype(np.float32),
    ),
)

_NC_CACHE = []


def _build_nc() -> bass.Bass:
    nc = bacc.Bacc("TRN2", target_bir_lowering=False, debug=False)
    # host pre-transposed: partition-major [b*c, ri, H, W] so every DMA is a
    # single-dim 128-partition transfer (hits all 16 SBUF AXI ports)
    x = nc.dram_tensor("x", [P, RI, H, W], F32, kind="ExternalInput").ap()
    out = nc.dram_tensor("out", [P, RI, HO, WO], F32, kind="ExternalOutput").ap()

    with tile.TileContext(nc) as tc:
        with tc.tile_pool(name="p4", bufs=4) as p4, \
             tc.tile_pool(name="p3", bufs=3) as p3, \
             tc.tile_pool(name="p2", bufs=2) as p2, \
             tc.tile_pool(name="p1", bufs=1) as p1:

            xri_t, riH_t, cV_t, mH_t, stage_t = {}, {}, {}, {}, {}

            def dma_in(k):
                xri = p4.tile([P, RI * N], F32, tag="xri")
                xri_t[k] = xri
                nc.sync.dma_start(
                    out=xri.rearrange("p (ri f) -> p ri f", ri=RI),
                    in_=x[:, :, k * RD : (k + 1) * RD, :].rearrange(
                        "p ri r w -> p ri (r w)"
                    ),
                )

            def xri6(k):
                return xri_t[k].rearrange(
                    "p (ri r w t) -> p ri r w t", ri=RI, r=RD, w=WO, t=2
                )

            def riH4(k):
                return riH_t[k].rearrange(
                    "p (ri r w) -> p ri r w", ri=RI, r=RD, w=WO
                )

            def riH5(k):
                return riH_t[k].rearrange(
                    "p (ri rp rt w) -> p ri rp rt w", ri=RI, rp=RO, rt=2, w=WO
                )

            def stage_dst4(k):
                g = k % GROUP
                return stage_t[k // GROUP].rearrange(
                    "p (ri r w) -> p ri r w", ri=RI, r=GROUP * RO, w=WO
                )[:, :, g * RO : (g + 1) * RO, :]

            for k in range(NCHUNK):
                if k == 0:
                    dma_in(0)
                    dma_in(1)
                if k + 2 < NCHUNK:
                    dma_in(k + 2)

                # ACT: prefill H-losers (odd cols) straight from the input
                riH = p3.tile([P, RI * RD * WO], F32, tag="riH")
                riH_t[k] = riH
                nc.scalar.copy(out=riH4(k), in_=xri6(k)[:, :, :, :, 1])

                # DVE: nrm = re^2 + im^2 (fused, one pass over the chunk)
                nrm = p1.tile([P, N], F32, tag="nrm")
                xrr = xri_t[k].rearrange("p (ri f) -> p ri f", ri=RI)
                nc.vector._custom_dve(
                    SQADD, out=nrm[:, :], in0=xrr[:, 0, :], in1=xrr[:, 1, :]
                )

                # DVE: smax = sel(nE>=nO, -nE, nO): sign=mask, |.|=H-max
                smax = p2.tile([P, RD * WO], F32, tag="smax")
                nrm_t = nrm.rearrange("p (x t) -> p x t", t=2)
                nc.vector._custom_dve(
                    SIGNSEL, out=smax[:, :], in0=nrm_t[:, :, 0], in1=nrm_t[:, :, 1]
                )

                # ACT: mH u8 = relu(sign(-smax)) in {0,1}; 1 <=> even/left won
                mHs = p1.tile([P, RD * WO], F32, tag="mHs")
                nc.scalar.activation(out=mHs, in_=smax, func=ACTF.Sign, scale=-1.0)
                mH = p2.tile([P, RD * WO], U8, tag="mH")
                mH_t[k] = mH
                nc.scalar.activation(out=mH, in_=mHs, func=ACTF.Relu)

                # DVE: cV = sq(smaxT) >= sq(smaxB)  (|smaxT| >= |smaxB|)
                cV = p3.tile([P, RO * WO], U8, tag="cV")
                cV_t[k] = cV
                sm5 = smax.rearrange(
                    "p (rp rt w) -> p rp rt w", rp=RO, rt=2, w=WO
                )
                cv3 = cV.rearrange("p (r w) -> p r w", r=RO, w=WO)
                nc.vector._custom_dve(
                    SQGE, out=cv3, in0=sm5[:, :, 0, :], in1=sm5[:, :, 1, :]
                )

                def predh(j):
                    mb = mH_t[j].rearrange(
                        "p (r w) -> p r w", r=RD, w=WO
                    ).unsqueeze(1).broadcast_to([P, RI, RD, WO])
                    nc.vector.copy_predicated(
                        out=riH4(j), mask=mb, data=xri6(j)[:, :, :, :, 0]
                    )
                    del xri_t[j], mH_t[j]

                def vpre(j):
                    if j % GROUP == 0:
                        stage_t[j // GROUP] = p2.tile(
                            [P, RI * GROUP * RO * WO], F32, tag="stage",
                            name="stage",
                        )
                    nc.scalar.copy(out=stage_dst4(j), in_=riH5(j)[:, :, :, 1, :])

                def predv(j):
                    cb = cV_t[j].rearrange(
                        "p (r w) -> p r w", r=RO, w=WO
                    ).unsqueeze(1).broadcast_to([P, RI, RO, WO])
                    nc.vector.copy_predicated(
                        out=stage_dst4(j), mask=cb, data=riH5(j)[:, :, :, 0, :]
                    )
                    del riH_t[j], cV_t[j]
                    if j >= NCHUNK - 2:
                        # drain: flush each of the last two chunks immediately
                        g = j % GROUP
                        st = stage_t[j // GROUP].rearrange(
                            "p (ri r w) -> p ri r w", ri=RI, r=GROUP * RO, w=WO
                        )[:, :, g * RO : (g + 1) * RO, :]
                        nc.sync.dma_start(
                            out=out[:, :, j * RO : (j + 1) * RO, :].rearrange(
                                "p ri r w -> p ri (r w)"
                            ),
                            in_=st.rearrange("p ri r w -> p ri (r w)"),
                        )
                        if j % GROUP == GROUP - 1:
                            del stage_t[j // GROUP]
                    elif j % GROUP == GROUP - 1:
                        jj = j // GROUP
                        nc.sync.dma_start(
                            out=out[
                                :, :, jj * GROUP * RO : (jj + 1) * GROUP * RO, :
                            ].rearrange("p ri r w -> p ri (r w)"),
                            in_=stage_t[jj].rearrange("p (ri f) -> p ri f", ri=RI),
                        )
                        del stage_t[jj]

                if k >= 1:
                    predh(k - 1)
                    vpre(k - 1)
                if k >= 2:
                    predv(k - 2)

            predh(NCHUNK - 1)
            vpre(NCHUNK - 1)
            predv(NCHUNK - 2)
            predv(NCHUNK - 1)
    nc.compile()
    return nc


def get_nc() -> bass.Bass:
    if not _NC_CACHE:
        _NC_CACHE.append(_build_nc())
    return _NC_CACHE[0]


def kernel(x: np.ndarray, **run_kwargs) -> np.ndarray:
    nc = get_nc()
    xs = np.asarray(x, dtype=np.float32)
    assert xs.shape == (NCORES * B, RI, C, H, W), xs.shape
    # [16,2,64,H,W] -> per core [b,c,ri,H,W] flattened to [128,ri,H,W]
    xt = np.ascontiguousarray(xs.transpose(0, 2, 1, 3, 4))
    in_maps = [
        {"x": xt[B * i : B * (i + 1)].reshape(P, RI, H, W)} for i in range(NCORES)
    ]
    res = bass_utils.run_bass_kernel_spmd(
        nc, in_maps, core_ids=list(range(NCORES)), **run_kwargs
    )
    # per-core [128,ri,HO,WO] -> [b,c,ri,HO,WO] -> [b,ri,c,HO,WO]
    out = np.concatenate(
        [
            res.results[i]["out"].reshape(B, C, RI, HO, WO).transpose(0, 2, 1, 3, 4)
            for i in range(NCORES)
        ],
        axis=0,
    )
    if run_kwargs:
        kernel.last_results = res
    return np.ascontiguousarray(out)
